# revision 1
# baseline (speedup 1.0000x reference)
"""Trainium2 Bass kernel for the CrossAttention problem (self-contained).

Strategy: shard the N=4096 query rows across 8 cores (512 rows/core, both
batch elements). Everything is computed in transposed layout (features on
partitions, query rows on the free dim) so every matmul has a wide moving
operand:

  qT   = (scale*Wq)^T @ xT          [512, 1024]   (rows 0:512 uc, 512:1024 cond)
  kT   = Wk^T @ ctxKT               [512, 5*77]   (uc, c0..c3 contexts)
  v    = ctxVT_g^T @ Wv             [5][77, 512]
  simT = k_gh @ qT_h                [77, 512] per (group, head)
  E    = exp(simT)  (logits are small; no max-subtraction needed)
  Z    = ones^T @ E                 [1, 512] rows into a dense PSUM stack
  attn = E * bcast(1/Z)             (PE broadcast of the recip row)
  outT = v_gh^T @ attn  (accumulated over the 4 cond branches; uc separate)
  yT   = Wo^T @ out_mergedT + bo    [320, 1024]

The soft-mask scalar wmask = w_dot * (t/50*4.6) * max(sim_c[0]) couples all
cores: each core computes its local branch-0 max, an AllReduce(max) collective
combines them while the other 4 groups are processed, then branch 0 finishes.
"""

import sys

sys.path.insert(0, "/opt/trn_rl_repo")

import numpy as np

import concourse.bass as bass
import concourse.tile as tile
from concourse import bacc, bass_utils, mybir

# problem constants (hardcoded per the harness contract)
H, DH, L, C = 8, 64, 77, 4
N, DQ, DC, INNER = 4096, 320, 768, 512
N_CORES = 8
NS = N // N_CORES          # query rows per core per batch element
NSB = 2 * NS               # both batch elements
SCALE = DH ** -0.5
W_DOT, TOTAL_STEP, SCHED = 1.0, 50, 4.6

F32 = mybir.dt.float32
F32R = mybir.dt.float32r
F16 = mybir.dt.float16

# groups in context order: 0=uc, 1..4 = cond branches 0..3
G_UC = 0

LAST_RESULTS = None  # BassKernelResults of the most recent run (for test.py)
TRACE = False


def _r(ap):
    return ap.bitcast(F32R)


def build_kernel(wdotw: float):
    nc = bacc.Bacc("TRN2", target_bir_lowering=False, debug=False, num_devices=N_CORES)

    # ---- DRAM I/O ----
    d_xt = nc.dram_tensor("xt", [384, NSB], F16, kind="ExternalInput")
    d_wq = nc.dram_tensor("wq", [384, INNER], F16, kind="ExternalInput")  # pre-scaled
    d_wk = nc.dram_tensor("wk", [DC, INNER], F16, kind="ExternalInput")
    d_wv = nc.dram_tensor("wv", [DC, INNER], F16, kind="ExternalInput")
    d_wo = nc.dram_tensor("wo", [INNER, DQ], F16, kind="ExternalInput")
    d_bo = nc.dram_tensor("bo", [384], F32, kind="ExternalInput")
    d_ctxkt = nc.dram_tensor("ctxkt", [DC, 5 * L], F16, kind="ExternalInput")
    d_ctxvt = nc.dram_tensor("ctxvt", [DC, 5 * L], F16, kind="ExternalInput")
    d_aet = nc.dram_tensor("aet", [H, L, NS], F32, kind="ExternalInput")
    d_yt = nc.dram_tensor("yt", [DQ, NSB], F32, kind="ExternalOutput")

    with tile.TileContext(nc) as tc:
        _emit(nc, tc, wdotw, d_xt, d_wq, d_wk, d_wv, d_wo, d_bo,
              d_ctxkt, d_ctxvt, d_aet, d_yt)
    nc.compile()
    return nc


def _emit(nc, tc, wdotw, d_xt, d_wq, d_wk, d_wv, d_wo, d_bo,
          d_ctxkt, d_ctxvt, d_aet, d_yt):
    from contextlib import ExitStack

    ctx = ExitStack()
    singles = ctx.enter_context(tc.tile_pool(name="singles", bufs=1))
    dram = ctx.enter_context(tc.tile_pool(name="dram", bufs=1, space="DRAM"))
    work = ctx.enter_context(tc.tile_pool(name="work", bufs=3))
    epool = ctx.enter_context(tc.tile_pool(name="epool", bufs=10))
    apool = ctx.enter_context(tc.tile_pool(name="apool", bufs=26))
    rzpool = ctx.enter_context(tc.tile_pool(name="rzpool", bufs=4))

    # ---- persistent SBUF tiles ----
    s_xt = singles.tile([128, 3, NSB], F16)
    s_wq = singles.tile([128, 3, INNER], F16)
    s_wk = singles.tile([128, 6, INNER], F16)
    s_wv = singles.tile([128, 6, INNER], F16)
    s_wo = singles.tile([128, 4, DQ], F16)
    s_bo = singles.tile([128, 3], F32)
    s_ctxkt = singles.tile([128, 6, 5 * L], F16)
    s_ctxvt = singles.tile([128, 6, 5 * L], F16)
    s_aet = singles.tile([L, H, NS], F32)
    s_qt = singles.tile([128, 4, NSB], F16)
    s_kt = singles.tile([128, 4, 5 * L], F16)
    s_ktc0 = singles.tile([128, 4, L], F16)
    s_vp = singles.tile([L, 5, INNER], F16)
    s_sc0 = singles.tile([L, H, NS], F32)       # branch-0 sims parked pre-mask
    s_om = singles.tile([128, 4, NSB], F16)     # merged outT (inner on partitions)
    s_y = singles.tile([128, 3, NSB], F32)
    s_lmax = singles.tile([L, H], F32)
    s_lm = singles.tile([L, 1], F32)
    s_maxrow8 = singles.tile([1, N_CORES * L], F32)
    s_wm = singles.tile([1, 1], F16)
    s_wmcol = singles.tile([L, 1], F32)
    ones77 = singles.tile([L, L], F16)
    ones_row = singles.tile([1, 128], F16)

    # ---- critical-path input DMA (collective prerequisites first) ----
    nc.sync.dma_start(out=s_xt[:], in_=d_xt.ap().rearrange("(c p) f -> p c f", p=128))
    nc.sync.dma_start(out=s_wq[:], in_=d_wq.ap().rearrange("(c p) f -> p c f", p=128))
    nc.sync.dma_start(out=s_wk[:], in_=d_wk.ap().rearrange("(c p) f -> p c f", p=128))
    nc.sync.dma_start(out=s_ctxkt[:], in_=d_ctxkt.ap().rearrange("(c p) f -> p c f", p=128))

    nc.vector.memset(ones77[:], 1.0)
    nc.vector.memset(ones_row[:], 1.0)

    psim = ctx.enter_context(tc.tile_pool(name="psim", bufs=2, space="PSUM"))
    pproj_cm = tc.tile_pool(name="pproj", bufs=2, space="PSUM")
    pproj = pproj_cm.__enter__()

    def qproj(half):
        for dc in range(4):
            p = pproj.tile([128, NS], F32, tag="proj")
            for kc in range(3):
                nc.tensor.matmul(
                    p[:],
                    s_wq[:, kc, dc * 128:(dc + 1) * 128],
                    s_xt[:, kc, half * NS:(half + 1) * NS],
                    start=(kc == 0), stop=(kc == 2),
                )
            nc.scalar.copy(s_qt[:, dc, half * NS:(half + 1) * NS], p[:])

    # ---- phase 0: just enough for the branch-0 max -> collective ----
    qproj(1)                                     # cond-half qT
    for dc in range(4):                          # branch-0 kT slice
        p = pproj.tile([128, 5 * L], F32, tag="proj")
        for kc in range(6):
            nc.tensor.matmul(
                p[0:128, 0:L],
                s_wk[:, kc, dc * 128:(dc + 1) * 128],
                s_ctxkt[:, kc, L:2 * L],
                start=(kc == 0), stop=(kc == 5),
            )
        nc.scalar.copy(s_ktc0[:, dc, :], p[0:128, 0:L])

    def qk0(h, psum_slice):
        nc.tensor.matmul(
            psum_slice,
            s_ktc0[(h % 2) * 64:(h % 2) * 64 + 64, h // 2, :],
            s_qt[(h % 2) * 64:(h % 2) * 64 + 64, h // 2, NS:NSB],
            start=True, stop=True,
        )

    for hp in range(4):
        p = psim.tile([L, 2, NS], F32, tag="sim")
        qk0(2 * hp, p[:, 0, :])
        qk0(2 * hp + 1, p[:, 1, :])
        nc.vector.reduce_max(out=s_lmax[:, 2 * hp:2 * hp + 2], in_=p[:],
                             axis=mybir.AxisListType.X)
        nc.scalar.copy(s_sc0[:, 2 * hp:2 * hp + 2, :], p[:])
    nc.vector.reduce_max(out=s_lm[:], in_=s_lmax[:], axis=mybir.AxisListType.X)
    nc.vector.tensor_scalar_mul(s_lm[:], s_lm[:], float(wdotw))

    cin = dram.tile([1, L], F32)
    cout = dram.tile([N_CORES, L], F32)
    nc.sync.dma_start(out=cin.rearrange("one f -> f one"), in_=s_lm[:])
    nc.gpsimd.collective_compute(
        "AllGather", mybir.AluOpType.bypass,
        replica_groups=[list(range(N_CORES))],
        ins=[cin.opt()], outs=[cout.opt()],
    )

    # ---- remaining input DMA ----
    nc.sync.dma_start(out=s_wv[:], in_=d_wv.ap().rearrange("(c p) f -> p c f", p=128))
    nc.sync.dma_start(out=s_ctxvt[:], in_=d_ctxvt.ap().rearrange("(c p) f -> p c f", p=128))
    nc.sync.dma_start(out=s_wo[:], in_=d_wo.ap().rearrange("(c p) f -> p c f", p=128))
    nc.sync.dma_start(out=s_bo[:], in_=d_bo.ap().rearrange("(c p) -> p c", p=128))
    nc.sync.dma_start(out=s_aet[:], in_=d_aet.ap().rearrange("h p f -> p h f"))

    # ---- phase 1: remaining projections ----
    qproj(0)                                     # uc-half qT
    for dc in range(4):                          # full kT (branch-0 cols unused)
        p = pproj.tile([128, 5 * L], F32, tag="proj")
        for kc in range(6):
            nc.tensor.matmul(
                p[:],
                s_wk[:, kc, dc * 128:(dc + 1) * 128],
                s_ctxkt[:, kc, :],
                start=(kc == 0), stop=(kc == 5),
            )
        nc.scalar.copy(s_kt[:, dc, :], p[:])
    for g in range(5):                           # v, with 1/C folded into cond
        p = pproj.tile([128, INNER], F32, tag="proj")
        for kc in range(6):
            nc.tensor.matmul(
                p[0:L, :],
                s_ctxvt[:, kc, g * L:(g + 1) * L],
                s_wv[:, kc, :],
                start=(kc == 0), stop=(kc == 5),
            )
        if g == G_UC:
            nc.scalar.copy(s_vp[:, g, :], p[0:L, :])
        else:
            nc.scalar.mul(s_vp[:, g, :], p[0:L, :], 1.0 / C)
    pproj_cm.__exit__(None, None, None)

    pzb = ctx.enter_context(tc.tile_pool(name="pzb", bufs=1, space="PSUM"))
    ppv = ctx.enter_context(tc.tile_pool(name="ppv", bufs=2, space="PSUM"))

    def qk(g, h, psum_slice):
        cols = slice(0, NS) if g == G_UC else slice(NS, NSB)
        nc.tensor.matmul(
            psum_slice,
            s_kt[(h % 2) * 64:(h % 2) * 64 + 64, h // 2, g * L:(g + 1) * L],
            s_qt[(h % 2) * 64:(h % 2) * 64 + 64, h // 2, cols],
            start=True, stop=True,
        )

    # ---- phase 3: uc + branches 1..3 (unit pairs) ----
    attn_c = {}
    anchors = {}

    def unit_pair(e_pair):
        zb = pzb.tile([L, 2, NS], F32, tag="zb")
        nc.tensor.matmul(zb[:, 0, :], ones77[:], e_pair[:, 0, :], start=True, stop=True)
        anchors["zb"] = nc.tensor.matmul(
            zb[:, 1, :], ones77[:], e_pair[:, 1, :], start=True, stop=True)
        rz = rzpool.tile([L, 2, NS], F32, tag="rz")
        nc.vector.reciprocal_approx_fast(out=rz[:], in_=zb[:])
        return rz

    for g in (0, 2, 3, 4):
        sims = []
        for hp in range(4):
            p = psim.tile([L, 2, NS], F32, tag="sim")
            qk(g, 2 * hp, p[:, 0, :])
            qk(g, 2 * hp + 1, p[:, 1, :])
            sims.append(p)
        for hp in range(4):
            e = epool.tile([L, 2, NS], F16, tag="e")
            anchors["exp"] = nc.scalar.activation(
                e[:], sims[hp][:], mybir.ActivationFunctionType.Exp)
            rz = unit_pair(e)
            for k in range(2):
                h = 2 * hp + k
                if g == G_UC:
                    pv = ppv.tile([64, NS], F32, tag="pv")
                    nc.tensor.matmul(pv[:], s_vp[:, 0, h * 64:(h + 1) * 64],
                                     e[:, k, :], start=True, stop=True)
                    nc.vector.tensor_mul(
                        s_om[(h % 2) * 64:(h % 2) * 64 + 64, h // 2, 0:NS],
                        pv[:], rz[0:64, k, :])
                else:
                    a = apool.tile([L, NS], F16, tag="attn")
                    anchors["mul"] = nc.vector.tensor_mul(a[:], e[:, k, :], rz[:, k, :])
                    attn_c[(g, h)] = a

    # ---- uc half of the output projection (independent of branch 0) ----
    def wo_half(half, pool):
        for oc in range(3):
            ow = 128 if oc < 2 else 64
            p = pool.tile([128, NS], F32, tag="pv")
            for kc in range(4):
                nc.tensor.matmul(
                    p[0:ow, :],
                    s_wo[:, kc, oc * 128:oc * 128 + ow],
                    s_om[:, kc, half * NS:(half + 1) * NS],
                    start=(kc == 0), stop=(kc == 3),
                )
            nc.scalar.add(s_y[0:ow, oc, half * NS:(half + 1) * NS], p[0:ow, :],
                          s_bo[0:ow, oc:oc + 1])
        for oc in range(3):
            ow = 128 if oc < 2 else 64
            nc.sync.dma_start(
                out=d_yt.ap()[oc * 128:oc * 128 + ow, half * NS:(half + 1) * NS],
                in_=s_y[0:ow, oc, half * NS:(half + 1) * NS])

    wo_half(0, ppv)

    # ---- phase 4: wmask from the gathered maxima, branch 0, PV chains ----
    nc.sync.dma_start(out=s_maxrow8[:], in_=cout.rearrange("r f -> (r f)"))
    red = nc.vector.reduce_max(out=s_wm[:], in_=s_maxrow8[:], axis=mybir.AxisListType.X)
    tile.add_dep_helper(red.ins, anchors["mul"].ins, sync=False,
                        reason="defer wmask path behind group work")
    p_wm = pzb.tile([L, 2, NS], F32, tag="zb")
    bc = nc.tensor.matmul(p_wm[:, 0, 0:1], ones_row[0:1, 0:L], s_wm[:],
                          start=True, stop=True)
    tile.add_dep_helper(bc.ins, anchors["zb"].ins, sync=False,
                        reason="defer wmask bcast behind group matmuls")
    nc.vector.tensor_copy(s_wmcol[:], p_wm[:, 0, 0:1])

    first_p4_exp = None
    for hp in range(4):
        msk = work.tile([L, 2, NS], F32, tag="msk")
        nc.vector.scalar_tensor_tensor(
            out=msk[:], in0=s_aet[:, 2 * hp:2 * hp + 2, :], scalar=s_wmcol[:],
            in1=s_sc0[:, 2 * hp:2 * hp + 2, :],
            op0=mybir.AluOpType.mult, op1=mybir.AluOpType.add,
        )
        e = epool.tile([L, 2, NS], F16, tag="e")
        ei = nc.scalar.activation(e[:], msk[:], mybir.ActivationFunctionType.Exp)
        if first_p4_exp is None:
            first_p4_exp = ei
            tile.add_dep_helper(ei.ins, anchors["exp"].ins, sync=False,
                                reason="defer branch-0 exp behind group exps")
        rz = unit_pair(e)
        for k in range(2):
            h = 2 * hp + k
            a = apool.tile([L, NS], F16, tag="attn")
            nc.vector.tensor_mul(a[:], e[:, k, :], rz[:, k, :])
            attn_c[(1, h)] = a
        for k in range(2):
            h = 2 * hp + k
            pv = ppv.tile([64, NS], F32, tag="pv")
            for i, g in enumerate((1, 2, 3, 4)):
                nc.tensor.matmul(pv[:], s_vp[:, g, h * 64:(h + 1) * 64],
                                 attn_c.pop((g, h))[:], start=(i == 0), stop=(i == 3))
            nc.scalar.copy(s_om[(h % 2) * 64:(h % 2) * 64 + 64, h // 2, NS:NSB], pv[:])

    # ---- phase 5: cond half of the output projection ----
    wo_half(1, ppv)
    ctx.pop_all().close()


_CACHE = {}


def kernel(x, uc_context, ck, cv, attn_extra, Wq, Wk, Wv, Wo, bo, t):
    global LAST_RESULTS
    x = np.ascontiguousarray(np.asarray(x, np.float32))
    uc_context = np.asarray(uc_context, np.float32)
    ck = np.asarray(ck, np.float32)
    cv = np.asarray(cv, np.float32)
    attn_extra = np.asarray(attn_extra, np.float32)
    Wq = np.asarray(Wq, np.float32)
    Wk = np.asarray(Wk, np.float32)
    Wv = np.asarray(Wv, np.float32)
    Wo = np.asarray(Wo, np.float32)
    bo = np.asarray(bo, np.float32)
    tv = float(np.asarray(t))
    wdotw = W_DOT * (tv / TOTAL_STEP) * SCHED

    if wdotw not in _CACHE:
        _CACHE[wdotw] = build_kernel(wdotw)
    nc = _CACHE[wdotw]

    # host-side input prep (layout only)
    wq_pad = np.zeros((384, INNER), np.float16)
    wq_pad[:DQ] = (Wq * SCALE).astype(np.float16)
    bo_pad = np.zeros((384,), np.float32)
    bo_pad[:DQ] = bo
    wk16 = Wk.astype(np.float16)
    wv16 = Wv.astype(np.float16)
    wo16 = Wo.astype(np.float16)
    ctxK = np.concatenate([uc_context[0][None], ck[:, 0]], axis=0)  # [5, 77, 768]
    ctxV = np.concatenate([uc_context[0][None], cv[:, 0]], axis=0)
    ctxkt = np.ascontiguousarray(ctxK.transpose(2, 0, 1).reshape(DC, 5 * L)).astype(np.float16)
    ctxvt = np.ascontiguousarray(ctxV.transpose(2, 0, 1).reshape(DC, 5 * L)).astype(np.float16)

    in_maps = []
    for c in range(N_CORES):
        rows = slice(c * NS, (c + 1) * NS)
        xt = np.zeros((384, NSB), np.float16)
        xt[:DQ, :NS] = x[0, rows].T.astype(np.float16)
        xt[:DQ, NS:] = x[1, rows].T.astype(np.float16)
        aet = np.ascontiguousarray(attn_extra[:, rows, :].transpose(0, 2, 1))
        in_maps.append({
            "xt": xt, "wq": wq_pad, "wk": wk16, "wv": wv16, "wo": wo16, "bo": bo_pad,
            "ctxkt": ctxkt, "ctxvt": ctxvt, "aet": aet,
        })

    import os as _os
    _tc = None
    if _os.environ.get("KERNEL_TRACE_ALL") == "1":
        _tc = list(range(N_CORES))
    res = bass_utils.run_bass_kernel_spmd(
        nc, in_maps, core_ids=list(range(N_CORES)), trace=TRACE, trace_cores=_tc,
    )
    LAST_RESULTS = res

    out = np.empty((2, N, DQ), np.float32)
    for c in range(N_CORES):
        rows = slice(c * NS, (c + 1) * NS)
        yt = res.results[c]["yt"]
        out[0, rows] = yt[:, :NS].T
        out[1, rows] = yt[:, NS:].T
    return out



# revision 5
# speedup vs baseline: 1.0778x; 1.0778x over previous
"""Trainium2 Bass kernel for the CrossAttention problem (self-contained).

Strategy: shard the N=4096 query rows across 8 cores (512 rows/core, both
batch elements). Everything is computed in transposed layout (features on
partitions, query rows on the free dim) so every matmul has a wide moving
operand:

  qT   = (scale*Wq)^T @ xT          [512, 1024]   (rows 0:512 uc, 512:1024 cond)
  kT   = Wk^T @ ctxKT               [512, 5*77]   (uc, c0..c3 contexts)
  v    = ctxVT_g^T @ Wv             [5][77, 512]
  simT = k_gh @ qT_h                [77, 512] per (group, head)
  E    = exp(simT)  (logits are small; no max-subtraction needed)
  Z    = ones77^T @ E               [77, 512] bcast rows, written back into the
                                    same PSUM tile the sims came from
  attn = RECIP1_MUL(Z, E)           one fused custom-DVE op: E * ~(1/Z)
                                    (exponent-flip seed + 1 Newton step, consts
                                    minimax-tuned; ~1.7e-3 rel err)
  outT = v_gh^T @ attn  (accumulated over the 4 cond branches; uc separate)
  yT   = Wo^T @ out_mergedT + bo    [320, 1024]

The soft-mask scalar wmask = w_dot * (t/50*4.6) * max(sim_c[0]) couples all
cores: each core computes its local branch-0 max, an AllGather collective
combines them while the other 4 groups are processed, then branch 0 finishes.
"""

import sys

sys.path.insert(0, "/opt/trn_rl_repo")

import numpy as np

import concourse.bass as bass
import concourse.tile as tile
from concourse import bacc, bass_utils, mybir
from concourse import dve_ops
from concourse.dve_spec import AluOp, Bin, Spec, Src0, Src1, C0, C1, C2, lower, _has_src1
from concourse.dve_uop import DveOpSpec

# ---- custom fused DVE op: out = Src1 * recip1NR(Src0) * C2 -----------------
_not_x = Bin(AluOp.BITWISE_NOT, Src0, Src0)
_y0 = _not_x * C0
_y1 = _y0 * (C1 - Src0 * _y0)


def _ref_recip1_mul(in0, in1, s0, s1, imm2):
    not_x = (~in0.view(np.int32)).view(np.float32)
    y0 = not_x * s0
    y1 = y0 * (s1 - in0 * y0)
    return in1 * y1


def _register_recip1_mul():
    for op in dve_ops.OPS:
        if op.name == "RECIP1_MUL_ANT":
            return op
    op = dve_ops.DveOp(
        "RECIP1_MUL_ANT",
        Spec(body=Src1 * _y1, reference=_ref_recip1_mul),
        subdim=False,
        uops_sha={},
    )
    dve_ops.OPS.append(op)
    dve_ops._SUB_OPCODE_FOR_NAME[op.name] = (
        dve_ops._CUSTOM_DVE_ROW_BASE + len(dve_ops.OPS) - 1)
    assert max(dve_ops._SUB_OPCODE_FOR_NAME.values()) < 0x20
    for ver in ("v3", "v4"):
        res = DveOpSpec(name=op.name, opcode=dve_ops.get_dve_sub_opcode(op.name),
                        uops=lower(op.spec, ver=ver), rd1_en=_has_src1(op.spec))
        op.uops_sha[ver] = res.sha(ver)
    return op


RECIP1_MUL = _register_recip1_mul()
# minimax constants for 1/x (octave-periodic, range-insensitive); the output
# scale of the 3-param fit is folded into c0/c1 (c' = sqrt(s)*c)
RC0, RC1 = -0.23549776, 2.00173237

# problem constants (hardcoded per the harness contract)
H, DH, L, C = 8, 64, 77, 4
N, DQ, DC, INNER = 4096, 320, 768, 512
N_CORES = 8
NS = N // N_CORES          # query rows per core per batch element
NSB = 2 * NS               # both batch elements
SCALE = DH ** -0.5
W_DOT, TOTAL_STEP, SCHED = 1.0, 50, 4.6

F32 = mybir.dt.float32
F16 = mybir.dt.float16

G_UC = 0                   # groups in context order: 0=uc, 1..4 = cond branches

LAST_RESULTS = None        # BassKernelResults of the most recent run (for test.py)
TRACE = False


def build_kernel(wdotw: float):
    nc = bacc.Bacc("TRN2", target_bir_lowering=False, debug=False, num_devices=N_CORES)

    d_xt = nc.dram_tensor("xt", [384, NSB], F16, kind="ExternalInput")
    d_wq = nc.dram_tensor("wq", [384, INNER], F16, kind="ExternalInput")  # pre-scaled
    d_wk = nc.dram_tensor("wk", [DC, INNER], F16, kind="ExternalInput")
    d_wv = nc.dram_tensor("wv", [DC, INNER], F16, kind="ExternalInput")
    d_wo = nc.dram_tensor("wo", [INNER, DQ], F16, kind="ExternalInput")
    d_bo = nc.dram_tensor("bo", [384], F32, kind="ExternalInput")
    d_ctxkt = nc.dram_tensor("ctxkt", [DC, 5 * L], F16, kind="ExternalInput")
    d_ctxvt = nc.dram_tensor("ctxvt", [DC, 5 * L], F16, kind="ExternalInput")
    d_aet = nc.dram_tensor("aet", [H, L, NS], F16, kind="ExternalInput")
    d_yt = nc.dram_tensor("yt", [DQ, NSB], F16, kind="ExternalOutput")

    with tile.TileContext(nc) as tc:
        _emit(nc, tc, wdotw, d_xt, d_wq, d_wk, d_wv, d_wo, d_bo,
              d_ctxkt, d_ctxvt, d_aet, d_yt)
    nc.compile()
    return nc


def _emit(nc, tc, wdotw, d_xt, d_wq, d_wk, d_wv, d_wo, d_bo,
          d_ctxkt, d_ctxvt, d_aet, d_yt):
    from contextlib import ExitStack

    ctx = ExitStack()
    singles = ctx.enter_context(tc.tile_pool(name="singles", bufs=1))
    dram = ctx.enter_context(tc.tile_pool(name="dram", bufs=1, space="DRAM"))
    epool = ctx.enter_context(tc.tile_pool(name="epool", bufs=22))
    mpool = ctx.enter_context(tc.tile_pool(name="mpool", bufs=3))

    # ---- persistent SBUF tiles ----
    s_xt = singles.tile([128, 3, NSB], F16)
    s_wq = singles.tile([128, 3, INNER], F16)
    s_wk = singles.tile([128, 6, INNER], F16)
    s_wv = singles.tile([128, 6, INNER], F16)
    s_wo = singles.tile([128, 4, DQ], F16)
    s_bo = singles.tile([128, 3], F32)
    s_ctxkt = singles.tile([128, 6, 5 * L], F16)
    s_ctxvt = singles.tile([128, 6, 5 * L], F16)
    s_aet = singles.tile([L, H, NS], F16)
    s_qt = singles.tile([128, 4, NSB], F16)
    s_kt = singles.tile([128, 4, 5 * L], F16)
    s_vp = singles.tile([L, 5, INNER], F16)
    s_sc0 = singles.tile([L, H, NS], F16)       # branch-0 sims parked pre-mask
    s_om = singles.tile([128, 4, NSB], F16)     # merged outT (inner on partitions)
    s_y = singles.tile([128, 3, NSB], F16)
    s_lmax = singles.tile([L, H], F32)
    s_lm = singles.tile([L, 1], F32)
    s_maxrow8 = singles.tile([1, N_CORES * L], F32)
    s_wm = singles.tile([1, 1], F16)
    s_wmcol = singles.tile([L, 1], F32)
    ones77 = singles.tile([L, L], F16)
    ones_row = singles.tile([1, 128], F16)

    # ---- critical-path input DMA (collective prerequisites first) ----
    nc.sync.dma_start(out=s_xt[:], in_=d_xt.ap().rearrange("(c p) f -> p c f", p=128))
    nc.sync.dma_start(out=s_wq[:], in_=d_wq.ap().rearrange("(c p) f -> p c f", p=128))
    nc.sync.dma_start(out=s_wk[:], in_=d_wk.ap().rearrange("(c p) f -> p c f", p=128))
    nc.sync.dma_start(out=s_ctxkt[:], in_=d_ctxkt.ap().rearrange("(c p) f -> p c f", p=128))

    nc.vector.memset(ones77[:], 1.0)
    nc.vector.memset(ones_row[:], 1.0)

    # PSUM: psim bufs=3 x [77,2,512]f32 (2 banks each) + pgen bufs=2 x
    # [128,512]f32 (1 bank each) = exactly 8 banks.
    psim = ctx.enter_context(tc.tile_pool(name="psim", bufs=3, space="PSUM"))
    pgen = ctx.enter_context(tc.tile_pool(name="pgen", bufs=2, space="PSUM"))

    def qproj(half):
        for dc in range(4):
            p = pgen.tile([128, NS], F32, tag="gen")
            for kc in range(3):
                nc.tensor.matmul(
                    p[:],
                    s_wq[:, kc, dc * 128:(dc + 1) * 128],
                    s_xt[:, kc, half * NS:(half + 1) * NS],
                    start=(kc == 0), stop=(kc == 2),
                )
            nc.scalar.copy(s_qt[:, dc, half * NS:(half + 1) * NS], p[:])

    # ---- phase 0: cond qT, full kT, branch-0 sims -> local max -> collective
    qproj(1)
    for dc in range(4):
        p = pgen.tile([128, NS], F32, tag="gen")
        for kc in range(6):
            nc.tensor.matmul(
                p[0:128, 0:5 * L],
                s_wk[:, kc, dc * 128:(dc + 1) * 128],
                s_ctxkt[:, kc, :],
                start=(kc == 0), stop=(kc == 5),
            )
        nc.scalar.copy(s_kt[:, dc, :], p[0:128, 0:5 * L])

    def qk(g, h, psum_slice):
        cols = slice(0, NS) if g == G_UC else slice(NS, NSB)
        nc.tensor.matmul(
            psum_slice,
            s_kt[(h % 2) * 64:(h % 2) * 64 + 64, h // 2, g * L:(g + 1) * L],
            s_qt[(h % 2) * 64:(h % 2) * 64 + 64, h // 2, cols],
            start=True, stop=True,
        )

    for hp in range(4):
        p = psim.tile([L, 2, NS], F32, tag="sim")
        qk(1, 2 * hp, p[:, 0, :])
        qk(1, 2 * hp + 1, p[:, 1, :])
        nc.vector.reduce_max(out=s_lmax[:, 2 * hp:2 * hp + 2], in_=p[:],
                             axis=mybir.AxisListType.X)
        nc.scalar.copy(s_sc0[:, 2 * hp:2 * hp + 2, :], p[:])
    nc.vector.reduce_max(out=s_lm[:], in_=s_lmax[:], axis=mybir.AxisListType.X)
    nc.vector.tensor_scalar_mul(s_lm[:], s_lm[:], float(wdotw))

    cin = dram.tile([1, L], F32)
    cout = dram.tile([N_CORES, L], F32)
    nc.sync.dma_start(out=cin.rearrange("one f -> f one"), in_=s_lm[:])
    nc.gpsimd.collective_compute(
        "AllGather", mybir.AluOpType.bypass,
        replica_groups=[list(range(N_CORES))],
        ins=[cin.opt()], outs=[cout.opt()],
    )

    # ---- remaining input DMA ----
    nc.sync.dma_start(out=s_wv[:], in_=d_wv.ap().rearrange("(c p) f -> p c f", p=128))
    nc.sync.dma_start(out=s_ctxvt[:], in_=d_ctxvt.ap().rearrange("(c p) f -> p c f", p=128))
    nc.sync.dma_start(out=s_wo[:], in_=d_wo.ap().rearrange("(c p) f -> p c f", p=128))
    nc.sync.dma_start(out=s_bo[:], in_=d_bo.ap().rearrange("(c p) -> p c", p=128))
    nc.sync.dma_start(out=s_aet[:], in_=d_aet.ap().rearrange("h p f -> p h f"))

    # ---- phase 1: uc qT + v projections ----
    qproj(0)
    for g in range(5):                           # v, with 1/C folded into cond
        p = pgen.tile([128, NS], F32, tag="gen")
        for kc in range(6):
            nc.tensor.matmul(
                p[0:L, :],
                s_ctxvt[:, kc, g * L:(g + 1) * L],
                s_wv[:, kc, :],
                start=(kc == 0), stop=(kc == 5),
            )
        if g == G_UC:
            nc.scalar.copy(s_vp[:, g, :], p[0:L, :])
        else:
            nc.scalar.mul(s_vp[:, g, :], p[0:L, :], 1.0 / C)

    # ---- phase 3: per-pair pipeline for groups uc, c1, c2, c3 -------------
    # Per pair: QK (PE, into psim slot) -> exp (ACT, psim->e f16) -> Z-matmul
    # (PE, ones77 @ e, written back INTO the same psim slot) -> fused
    # recip-mul (DVE, attn overwrites e in place).  QK runs 2 pairs ahead of
    # Z so the tensor engine never waits on the softmax chain.
    attn_c = {}
    anchors = {}

    def pair_front(g, hp):
        """QK for pair hp of group g -> returns psim tile."""
        p = psim.tile([L, 2, NS], F32, tag="sim")
        qk(g, 2 * hp, p[:, 0, :])
        qk(g, 2 * hp + 1, p[:, 1, :])
        return p

    def pair_mid(g, hp, p):
        """exp -> Z (back into p) -> fused recip-mul; returns attn (f16)."""
        e = epool.tile([L, 2, NS], F16, tag="e")
        anchors["exp"] = nc.scalar.activation(
            e[:], p[:], mybir.ActivationFunctionType.Exp)
        for k in range(2):
            anchors["z"] = nc.tensor.matmul(
                p[:, k, :], ones77[:], e[:, k, :], start=True, stop=True)
        anchors["rm"] = nc.vector._custom_dve(
            RECIP1_MUL, out=e[:], in0=p[:], in1=e[:], s0=RC0, s1=RC1)
        return e

    # s_om layout matches the baseline merge: head h occupies partitions
    # (h%2)*64..+64 of free-chunk h//2, so pair (2hp, 2hp+1) is exactly the
    # full 128 partitions of chunk hp — one copy per pair.
    def pv_pair(g_list, hp, a_by_g, cols, start, stop):
        pv = pgen.tile([128, NS], F32, tag="gen")
        for k in range(2):
            h = 2 * hp + k
            for i, g in enumerate(g_list):
                nc.tensor.matmul(pv[k * 64:k * 64 + 64, :],
                                 s_vp[:, g, h * 64:(h + 1) * 64],
                                 a_by_g[g][:, k, :],
                                 start=(i == 0), stop=(i == len(g_list) - 1))
        nc.scalar.copy(s_om[:, hp, cols], pv[:])

    # uc group first (PV immediate), then cond groups 2..4 (attn parked)
    fronts = {}
    order = [(0, hp) for hp in range(4)] + [(g, hp) for g in (2, 3, 4) for hp in range(4)]
    lead = 2
    for i, (g, hp) in enumerate(order):
        fronts[(g, hp)] = pair_front(g, hp)
        j = i - lead
        if j >= 0:
            gj, hpj = order[j]
            a = pair_mid(gj, hpj, fronts.pop((gj, hpj)))
            if gj == 0:
                pv_pair([0], hpj, {0: a}, slice(0, NS), True, True)
            else:
                attn_c[(gj, hpj)] = a
    for j in range(len(order) - lead, len(order)):
        gj, hpj = order[j]
        a = pair_mid(gj, hpj, fronts.pop((gj, hpj)))
        if gj == 0:
            pv_pair([0], hpj, {0: a}, slice(0, NS), True, True)
        else:
            attn_c[(gj, hpj)] = a

    # ---- uc half of the output projection (fills PE while collective lands)
    def wo_half(half):
        for oc in range(3):
            ow = 128 if oc < 2 else 64
            p = pgen.tile([128, NS], F32, tag="gen")
            for kc in range(4):
                nc.tensor.matmul(
                    p[0:ow, :],
                    s_wo[:, kc, oc * 128:oc * 128 + ow],
                    s_om[:, kc, half * NS:(half + 1) * NS],
                    start=(kc == 0), stop=(kc == 3),
                )
            nc.scalar.add(s_y[0:ow, oc, half * NS:(half + 1) * NS], p[0:ow, :],
                          s_bo[0:ow, oc:oc + 1])
        for oc in range(3):
            ow = 128 if oc < 2 else 64
            nc.sync.dma_start(
                out=d_yt.ap()[oc * 128:oc * 128 + ow, half * NS:(half + 1) * NS],
                in_=s_y[0:ow, oc, half * NS:(half + 1) * NS])

    wo_half(0)

    # ---- phase 4: wmask from the gathered maxima, branch 0, PV chains ----
    nc.sync.dma_start(out=s_maxrow8[:], in_=cout.rearrange("r f -> (r f)"))
    red = nc.vector.reduce_max(out=s_wm[:], in_=s_maxrow8[:], axis=mybir.AxisListType.X)
    tile.add_dep_helper(red.ins, anchors["rm"].ins, sync=False,
                        reason="defer wmask path behind group work")
    p_wm = psim.tile([L, 2, NS], F32, tag="sim")
    bc = nc.tensor.matmul(p_wm[:, 0, 0:1], ones_row[0:1, 0:L], s_wm[:],
                          start=True, stop=True)
    tile.add_dep_helper(bc.ins, anchors["z"].ins, sync=False,
                        reason="defer wmask bcast behind group matmuls")
    nc.vector.tensor_copy(s_wmcol[:], p_wm[:, 0, 0:1])

    b0 = {}
    first_p4_exp = [None]

    def b0_mid(hp):
        msk = mpool.tile([L, 2, NS], F16, tag="msk")
        nc.vector.scalar_tensor_tensor(
            out=msk[:], in0=s_aet[:, 2 * hp:2 * hp + 2, :], scalar=s_wmcol[:],
            in1=s_sc0[:, 2 * hp:2 * hp + 2, :],
            op0=mybir.AluOpType.mult, op1=mybir.AluOpType.add,
        )
        e = epool.tile([L, 2, NS], F16, tag="e")
        ei = nc.scalar.activation(e[:], msk[:], mybir.ActivationFunctionType.Exp)
        if first_p4_exp[0] is None:
            first_p4_exp[0] = ei
            tile.add_dep_helper(ei.ins, anchors["exp"].ins, sync=False,
                                reason="defer branch-0 exp behind group exps")
        p = psim.tile([L, 2, NS], F32, tag="sim")
        for k in range(2):
            nc.tensor.matmul(p[:, k, :], ones77[:], e[:, k, :], start=True, stop=True)
        nc.vector._custom_dve(
            RECIP1_MUL, out=e[:], in0=p[:], in1=e[:], s0=RC0, s1=RC1)
        return e

    for hp in range(4):
        b0[hp] = b0_mid(hp)
    for hp in range(4):
        amap = {1: b0[hp], 2: attn_c.pop((2, hp)), 3: attn_c.pop((3, hp)),
                4: attn_c.pop((4, hp))}
        pv_pair([1, 2, 3, 4], hp, amap, slice(NS, NSB), True, True)

    # ---- phase 5: cond half of the output projection ----
    wo_half(1)
    ctx.pop_all().close()


_CACHE = {}


def kernel(x, uc_context, ck, cv, attn_extra, Wq, Wk, Wv, Wo, bo, t):
    global LAST_RESULTS
    x = np.ascontiguousarray(np.asarray(x, np.float32))
    uc_context = np.asarray(uc_context, np.float32)
    ck = np.asarray(ck, np.float32)
    cv = np.asarray(cv, np.float32)
    attn_extra = np.asarray(attn_extra, np.float32)
    Wq = np.asarray(Wq, np.float32)
    Wk = np.asarray(Wk, np.float32)
    Wv = np.asarray(Wv, np.float32)
    Wo = np.asarray(Wo, np.float32)
    bo = np.asarray(bo, np.float32)
    tv = float(np.asarray(t))
    wdotw = W_DOT * (tv / TOTAL_STEP) * SCHED

    if wdotw not in _CACHE:
        _CACHE[wdotw] = build_kernel(wdotw)
    nc = _CACHE[wdotw]

    # host-side input prep (layout only)
    wq_pad = np.zeros((384, INNER), np.float16)
    wq_pad[:DQ] = (Wq * SCALE).astype(np.float16)
    bo_pad = np.zeros((384,), np.float32)
    bo_pad[:DQ] = bo
    wk16 = Wk.astype(np.float16)
    wv16 = Wv.astype(np.float16)
    wo16 = Wo.astype(np.float16)
    ctxK = np.concatenate([uc_context[0][None], ck[:, 0]], axis=0)  # [5, 77, 768]
    ctxV = np.concatenate([uc_context[0][None], cv[:, 0]], axis=0)
    ctxkt = np.ascontiguousarray(ctxK.transpose(2, 0, 1).reshape(DC, 5 * L)).astype(np.float16)
    ctxvt = np.ascontiguousarray(ctxV.transpose(2, 0, 1).reshape(DC, 5 * L)).astype(np.float16)

    in_maps = []
    for c in range(N_CORES):
        rows = slice(c * NS, (c + 1) * NS)
        xt = np.zeros((384, NSB), np.float16)
        xt[:DQ, :NS] = x[0, rows].T.astype(np.float16)
        xt[:DQ, NS:] = x[1, rows].T.astype(np.float16)
        aet = np.ascontiguousarray(
            attn_extra[:, rows, :].transpose(0, 2, 1)).astype(np.float16)
        in_maps.append({
            "xt": xt, "wq": wq_pad, "wk": wk16, "wv": wv16, "wo": wo16, "bo": bo_pad,
            "ctxkt": ctxkt, "ctxvt": ctxvt, "aet": aet,
        })

    import os as _os
    _tc = None
    if _os.environ.get("KERNEL_TRACE_ALL") == "1":
        _tc = list(range(N_CORES))
    res = bass_utils.run_bass_kernel_spmd(
        nc, in_maps, core_ids=list(range(N_CORES)), trace=TRACE, trace_cores=_tc,
    )
    LAST_RESULTS = res

    out = np.empty((2, N, DQ), np.float32)
    for c in range(N_CORES):
        rows = slice(c * NS, (c + 1) * NS)
        yt = res.results[c]["yt"].astype(np.float32)
        out[0, rows] = yt[:, :NS].T
        out[1, rows] = yt[:, NS:].T
    return out


# revision 6
# speedup vs baseline: 1.1108x; 1.0306x over previous
"""Trainium2 Bass kernel for the CrossAttention problem (self-contained).

Strategy: shard the N=4096 query rows across 8 cores (512 rows/core, both
batch elements). Everything is computed in transposed layout (features on
partitions, query rows on the free dim) so every matmul has a wide moving
operand:

  qT   = (scale*Wq)^T @ xT          [512, 1024]   (rows 0:512 uc, 512:1024 cond)
  kT   = Wk^T @ ctxKT               [512, 5*77]   (uc, c0..c3 contexts)
  v    = ctxVT_g^T @ Wv             [5][77, 512]
  simT = k_gh @ qT_h                [77, 512] per (group, head)
  E    = exp(simT)  (logits are small; no max-subtraction needed)
  Z    = ones77^T @ E               [77, 512] bcast rows, written back into the
                                    same PSUM tile the sims came from
  attn = RECIP1_MUL(Z, E)           one fused custom-DVE op: E * ~(1/Z)
                                    (exponent-flip seed + 1 Newton step, consts
                                    minimax-tuned; ~1.7e-3 rel err)
  outT = v_gh^T @ attn  (accumulated over the 4 cond branches; uc separate)
  yT   = Wo^T @ out_mergedT + bo    [320, 1024]

The soft-mask scalar wmask = w_dot * (t/50*4.6) * max(sim_c[0]) couples all
cores: each core computes its local branch-0 max, an AllGather collective
combines them while the other 4 groups are processed, then branch 0 finishes.
"""

import sys

sys.path.insert(0, "/opt/trn_rl_repo")

import numpy as np

import concourse.bass as bass
import concourse.tile as tile
from concourse import bacc, bass_utils, mybir
from concourse import dve_ops
from concourse.dve_spec import AluOp, Bin, Spec, Src0, Src1, C0, C1, C2, lower, _has_src1
from concourse.dve_uop import DveOpSpec

# ---- custom fused DVE op: out = Src1 * recip1NR(Src0) * C2 -----------------
_not_x = Bin(AluOp.BITWISE_NOT, Src0, Src0)
_y0 = _not_x * C0
_y1 = _y0 * (C1 - Src0 * _y0)


def _ref_recip1_mul(in0, in1, s0, s1, imm2):
    not_x = (~in0.view(np.int32)).view(np.float32)
    y0 = not_x * s0
    y1 = y0 * (s1 - in0 * y0)
    return in1 * y1


def _register_recip1_mul():
    for op in dve_ops.OPS:
        if op.name == "RECIP1_MUL_ANT":
            return op
    op = dve_ops.DveOp(
        "RECIP1_MUL_ANT",
        Spec(body=Src1 * _y1, reference=_ref_recip1_mul),
        subdim=False,
        uops_sha={},
    )
    dve_ops.OPS.append(op)
    dve_ops._SUB_OPCODE_FOR_NAME[op.name] = (
        dve_ops._CUSTOM_DVE_ROW_BASE + len(dve_ops.OPS) - 1)
    assert max(dve_ops._SUB_OPCODE_FOR_NAME.values()) < 0x20
    for ver in ("v3", "v4"):
        res = DveOpSpec(name=op.name, opcode=dve_ops.get_dve_sub_opcode(op.name),
                        uops=lower(op.spec, ver=ver), rd1_en=_has_src1(op.spec))
        op.uops_sha[ver] = res.sha(ver)
    return op


RECIP1_MUL = _register_recip1_mul()
# minimax constants for 1/x (octave-periodic, range-insensitive); the output
# scale of the 3-param fit is folded into c0/c1 (c' = sqrt(s)*c)
RC0, RC1 = -0.23549776, 2.00173237

# problem constants (hardcoded per the harness contract)
H, DH, L, C = 8, 64, 77, 4
N, DQ, DC, INNER = 4096, 320, 768, 512
N_CORES = 8
NS = N // N_CORES          # query rows per core per batch element
NSB = 2 * NS               # both batch elements
SCALE = DH ** -0.5
W_DOT, TOTAL_STEP, SCHED = 1.0, 50, 4.6

F32 = mybir.dt.float32
F16 = mybir.dt.float16

G_UC = 0                   # groups in context order: 0=uc, 1..4 = cond branches

LAST_RESULTS = None        # BassKernelResults of the most recent run (for test.py)
TRACE = False


def build_kernel(wdotw: float):
    nc = bacc.Bacc("TRN2", target_bir_lowering=False, debug=False, num_devices=N_CORES)

    d_xt = nc.dram_tensor("xt", [384, NSB], F16, kind="ExternalInput")
    d_wq = nc.dram_tensor("wq", [384, INNER], F16, kind="ExternalInput")  # pre-scaled
    d_wk = nc.dram_tensor("wk", [DC, INNER], F16, kind="ExternalInput")
    d_wv = nc.dram_tensor("wv", [DC, INNER], F16, kind="ExternalInput")
    d_wo = nc.dram_tensor("wo", [INNER, DQ], F16, kind="ExternalInput")
    d_bo = nc.dram_tensor("bo", [384], F32, kind="ExternalInput")
    d_ctxkt = nc.dram_tensor("ctxkt", [DC, 5 * L], F16, kind="ExternalInput")
    d_ctxvt = nc.dram_tensor("ctxvt", [DC, 5 * L], F16, kind="ExternalInput")
    d_aet = nc.dram_tensor("aet", [H, L, NS], F16, kind="ExternalInput")
    d_yt = nc.dram_tensor("yt", [DQ, NSB], F16, kind="ExternalOutput")

    with tile.TileContext(nc) as tc:
        _emit(nc, tc, wdotw, d_xt, d_wq, d_wk, d_wv, d_wo, d_bo,
              d_ctxkt, d_ctxvt, d_aet, d_yt)
    nc.compile()
    return nc


def _emit(nc, tc, wdotw, d_xt, d_wq, d_wk, d_wv, d_wo, d_bo,
          d_ctxkt, d_ctxvt, d_aet, d_yt):
    from contextlib import ExitStack

    ctx = ExitStack()
    singles = ctx.enter_context(tc.tile_pool(name="singles", bufs=1))
    dram = ctx.enter_context(tc.tile_pool(name="dram", bufs=1, space="DRAM"))
    epool = ctx.enter_context(tc.tile_pool(name="epool", bufs=22))
    mpool = ctx.enter_context(tc.tile_pool(name="mpool", bufs=3))

    # ---- persistent SBUF tiles ----
    s_xt = singles.tile([128, 3, NSB], F16)
    s_wq = singles.tile([128, 3, INNER], F16)
    s_wk = singles.tile([128, 6, INNER], F16)
    s_wv = singles.tile([128, 6, INNER], F16)
    s_wo = singles.tile([128, 4, DQ], F16)
    s_bo = singles.tile([128, 3], F32)
    s_ctxkt = singles.tile([128, 6, 5 * L], F16)
    s_ctxvt = singles.tile([128, 6, 5 * L], F16)
    s_aet = singles.tile([L, H, NS], F16)
    s_qt = singles.tile([128, 4, NSB], F16)
    s_kt = singles.tile([128, 4, 5 * L], F16)
    s_vp = singles.tile([L, 5, INNER], F16)
    s_sc0 = singles.tile([L, H, NS], F16)       # branch-0 sims parked pre-mask
    s_om = singles.tile([128, 4, NSB], F16)     # merged outT (inner on partitions)
    s_y = singles.tile([128, 3, NSB], F16)
    s_lmax = singles.tile([L, H], F32)
    s_lm = singles.tile([L, 1], F32)
    s_maxrow8 = singles.tile([1, N_CORES * L], F32)
    s_wm = singles.tile([1, 1], F16)
    s_wmcol = singles.tile([L, 1], F32)
    ones77 = singles.tile([L, L], F16)
    ones_row = singles.tile([1, 128], F16)

    # ---- CC-stream warm-up: a dummy 4-byte AllGather issued first thing so
    # the one-time collective barrier/init (~50-60us) overlaps phases 0-3
    # instead of serializing before the wmask AllGather.
    s_dummy = singles.tile([1, 1], F32)
    dumm_in = dram.tile([1, 1], F32)
    dumm_out = dram.tile([N_CORES, 1], F32)
    nc.vector.memset(s_dummy[:], 0.0)
    nc.sync.dma_start(out=dumm_in[:], in_=s_dummy[:])
    nc.gpsimd.collective_compute(
        "AllGather", mybir.AluOpType.bypass,
        replica_groups=[list(range(N_CORES))],
        ins=[dumm_in.opt()], outs=[dumm_out.opt()],
    )

    # ---- critical-path input DMA (collective prerequisites first) ----
    nc.sync.dma_start(out=s_xt[:], in_=d_xt.ap().rearrange("(c p) f -> p c f", p=128))
    nc.sync.dma_start(out=s_wq[:], in_=d_wq.ap().rearrange("(c p) f -> p c f", p=128))
    nc.sync.dma_start(out=s_wk[:], in_=d_wk.ap().rearrange("(c p) f -> p c f", p=128))
    nc.sync.dma_start(out=s_ctxkt[:], in_=d_ctxkt.ap().rearrange("(c p) f -> p c f", p=128))

    nc.vector.memset(ones77[:], 1.0)
    nc.vector.memset(ones_row[:], 1.0)

    # PSUM: psim bufs=3 x [77,2,512]f32 (2 banks each) + pgen bufs=2 x
    # [128,512]f32 (1 bank each) = exactly 8 banks.
    psim = ctx.enter_context(tc.tile_pool(name="psim", bufs=3, space="PSUM"))
    pgen = ctx.enter_context(tc.tile_pool(name="pgen", bufs=2, space="PSUM"))

    def qproj(half):
        for dc in range(4):
            p = pgen.tile([128, NS], F32, tag="gen")
            for kc in range(3):
                nc.tensor.matmul(
                    p[:],
                    s_wq[:, kc, dc * 128:(dc + 1) * 128],
                    s_xt[:, kc, half * NS:(half + 1) * NS],
                    start=(kc == 0), stop=(kc == 2),
                )
            nc.scalar.copy(s_qt[:, dc, half * NS:(half + 1) * NS], p[:])

    # ---- phase 0: cond qT, full kT, branch-0 sims -> local max -> collective
    qproj(1)
    for dc in range(4):
        p = pgen.tile([128, NS], F32, tag="gen")
        for kc in range(6):
            nc.tensor.matmul(
                p[0:128, 0:5 * L],
                s_wk[:, kc, dc * 128:(dc + 1) * 128],
                s_ctxkt[:, kc, :],
                start=(kc == 0), stop=(kc == 5),
            )
        nc.scalar.copy(s_kt[:, dc, :], p[0:128, 0:5 * L])

    def qk(g, h, psum_slice):
        cols = slice(0, NS) if g == G_UC else slice(NS, NSB)
        nc.tensor.matmul(
            psum_slice,
            s_kt[(h % 2) * 64:(h % 2) * 64 + 64, h // 2, g * L:(g + 1) * L],
            s_qt[(h % 2) * 64:(h % 2) * 64 + 64, h // 2, cols],
            start=True, stop=True,
        )

    for hp in range(4):
        p = psim.tile([L, 2, NS], F32, tag="sim")
        qk(1, 2 * hp, p[:, 0, :])
        qk(1, 2 * hp + 1, p[:, 1, :])
        nc.vector.reduce_max(out=s_lmax[:, 2 * hp:2 * hp + 2], in_=p[:],
                             axis=mybir.AxisListType.X)
        nc.scalar.copy(s_sc0[:, 2 * hp:2 * hp + 2, :], p[:])
    nc.vector.reduce_max(out=s_lm[:], in_=s_lmax[:], axis=mybir.AxisListType.X)
    nc.vector.tensor_scalar_mul(s_lm[:], s_lm[:], float(wdotw))

    cin = dram.tile([1, L], F32)
    cout = dram.tile([N_CORES, L], F32)
    nc.sync.dma_start(out=cin.rearrange("one f -> f one"), in_=s_lm[:])
    nc.gpsimd.collective_compute(
        "AllGather", mybir.AluOpType.bypass,
        replica_groups=[list(range(N_CORES))],
        ins=[cin.opt()], outs=[cout.opt()],
    )

    # ---- remaining input DMA ----
    nc.sync.dma_start(out=s_wv[:], in_=d_wv.ap().rearrange("(c p) f -> p c f", p=128))
    nc.sync.dma_start(out=s_ctxvt[:], in_=d_ctxvt.ap().rearrange("(c p) f -> p c f", p=128))
    nc.sync.dma_start(out=s_wo[:], in_=d_wo.ap().rearrange("(c p) f -> p c f", p=128))
    nc.sync.dma_start(out=s_bo[:], in_=d_bo.ap().rearrange("(c p) -> p c", p=128))
    nc.sync.dma_start(out=s_aet[:], in_=d_aet.ap().rearrange("h p f -> p h f"))

    # ---- phase 1: uc qT + v projections ----
    qproj(0)
    for g in range(5):                           # v, with 1/C folded into cond
        p = pgen.tile([128, NS], F32, tag="gen")
        for kc in range(6):
            nc.tensor.matmul(
                p[0:L, :],
                s_ctxvt[:, kc, g * L:(g + 1) * L],
                s_wv[:, kc, :],
                start=(kc == 0), stop=(kc == 5),
            )
        if g == G_UC:
            nc.scalar.copy(s_vp[:, g, :], p[0:L, :])
        else:
            nc.scalar.mul(s_vp[:, g, :], p[0:L, :], 1.0 / C)

    # ---- phase 3: per-pair pipeline for groups uc, c1, c2, c3 -------------
    # Per pair: QK (PE, into psim slot) -> exp (ACT, psim->e f16) -> Z-matmul
    # (PE, ones77 @ e, written back INTO the same psim slot) -> fused
    # recip-mul (DVE, attn overwrites e in place).  QK runs 2 pairs ahead of
    # Z so the tensor engine never waits on the softmax chain.
    attn_c = {}
    anchors = {}

    def pair_front(g, hp):
        """QK for pair hp of group g -> returns psim tile."""
        p = psim.tile([L, 2, NS], F32, tag="sim")
        qk(g, 2 * hp, p[:, 0, :])
        qk(g, 2 * hp + 1, p[:, 1, :])
        return p

    def pair_mid(g, hp, p):
        """exp -> Z (back into p) -> fused recip-mul; returns attn (f16)."""
        e = epool.tile([L, 2, NS], F16, tag="e")
        anchors["exp"] = nc.scalar.activation(
            e[:], p[:], mybir.ActivationFunctionType.Exp)
        for k in range(2):
            anchors["z"] = nc.tensor.matmul(
                p[:, k, :], ones77[:], e[:, k, :], start=True, stop=True)
        anchors["rm"] = nc.vector._custom_dve(
            RECIP1_MUL, out=e[:], in0=p[:], in1=e[:], s0=RC0, s1=RC1)
        return e

    # s_om layout matches the baseline merge: head h occupies partitions
    # (h%2)*64..+64 of free-chunk h//2, so pair (2hp, 2hp+1) is exactly the
    # full 128 partitions of chunk hp — one copy per pair.
    def pv_pair(g_list, hp, a_by_g, cols, start, stop):
        pv = pgen.tile([128, NS], F32, tag="gen")
        for k in range(2):
            h = 2 * hp + k
            for i, g in enumerate(g_list):
                nc.tensor.matmul(pv[k * 64:k * 64 + 64, :],
                                 s_vp[:, g, h * 64:(h + 1) * 64],
                                 a_by_g[g][:, k, :],
                                 start=(i == 0), stop=(i == len(g_list) - 1))
        nc.scalar.copy(s_om[:, hp, cols], pv[:])

    # uc group first (PV immediate), then cond groups 2..4 (attn parked)
    fronts = {}
    order = [(0, hp) for hp in range(4)] + [(g, hp) for g in (2, 3, 4) for hp in range(4)]
    lead = 2
    for i, (g, hp) in enumerate(order):
        fronts[(g, hp)] = pair_front(g, hp)
        j = i - lead
        if j >= 0:
            gj, hpj = order[j]
            a = pair_mid(gj, hpj, fronts.pop((gj, hpj)))
            if gj == 0:
                pv_pair([0], hpj, {0: a}, slice(0, NS), True, True)
            else:
                attn_c[(gj, hpj)] = a
    for j in range(len(order) - lead, len(order)):
        gj, hpj = order[j]
        a = pair_mid(gj, hpj, fronts.pop((gj, hpj)))
        if gj == 0:
            pv_pair([0], hpj, {0: a}, slice(0, NS), True, True)
        else:
            attn_c[(gj, hpj)] = a

    # ---- uc half of the output projection (fills PE while collective lands)
    def wo_half(half):
        for oc in range(3):
            ow = 128 if oc < 2 else 64
            p = pgen.tile([128, NS], F32, tag="gen")
            for kc in range(4):
                nc.tensor.matmul(
                    p[0:ow, :],
                    s_wo[:, kc, oc * 128:oc * 128 + ow],
                    s_om[:, kc, half * NS:(half + 1) * NS],
                    start=(kc == 0), stop=(kc == 3),
                )
            nc.scalar.add(s_y[0:ow, oc, half * NS:(half + 1) * NS], p[0:ow, :],
                          s_bo[0:ow, oc:oc + 1])
        for oc in range(3):
            ow = 128 if oc < 2 else 64
            nc.sync.dma_start(
                out=d_yt.ap()[oc * 128:oc * 128 + ow, half * NS:(half + 1) * NS],
                in_=s_y[0:ow, oc, half * NS:(half + 1) * NS])

    wo_half(0)

    # ---- phase 4: wmask from the gathered maxima, branch 0, PV chains ----
    nc.sync.dma_start(out=s_maxrow8[:], in_=cout.rearrange("r f -> (r f)"))
    red = nc.vector.reduce_max(out=s_wm[:], in_=s_maxrow8[:], axis=mybir.AxisListType.X)
    tile.add_dep_helper(red.ins, anchors["rm"].ins, sync=False,
                        reason="defer wmask path behind group work")
    p_wm = psim.tile([L, 2, NS], F32, tag="sim")
    bc = nc.tensor.matmul(p_wm[:, 0, 0:1], ones_row[0:1, 0:L], s_wm[:],
                          start=True, stop=True)
    tile.add_dep_helper(bc.ins, anchors["z"].ins, sync=False,
                        reason="defer wmask bcast behind group matmuls")
    nc.vector.tensor_copy(s_wmcol[:], p_wm[:, 0, 0:1])

    b0 = {}
    first_p4_exp = [None]

    def b0_mid(hp):
        msk = mpool.tile([L, 2, NS], F16, tag="msk")
        nc.vector.scalar_tensor_tensor(
            out=msk[:], in0=s_aet[:, 2 * hp:2 * hp + 2, :], scalar=s_wmcol[:],
            in1=s_sc0[:, 2 * hp:2 * hp + 2, :],
            op0=mybir.AluOpType.mult, op1=mybir.AluOpType.add,
        )
        e = epool.tile([L, 2, NS], F16, tag="e")
        ei = nc.scalar.activation(e[:], msk[:], mybir.ActivationFunctionType.Exp)
        if first_p4_exp[0] is None:
            first_p4_exp[0] = ei
            tile.add_dep_helper(ei.ins, anchors["exp"].ins, sync=False,
                                reason="defer branch-0 exp behind group exps")
        p = psim.tile([L, 2, NS], F32, tag="sim")
        for k in range(2):
            nc.tensor.matmul(p[:, k, :], ones77[:], e[:, k, :], start=True, stop=True)
        nc.vector._custom_dve(
            RECIP1_MUL, out=e[:], in0=p[:], in1=e[:], s0=RC0, s1=RC1)
        return e

    for hp in range(4):
        b0[hp] = b0_mid(hp)
    for hp in range(4):
        amap = {1: b0[hp], 2: attn_c.pop((2, hp)), 3: attn_c.pop((3, hp)),
                4: attn_c.pop((4, hp))}
        pv_pair([1, 2, 3, 4], hp, amap, slice(NS, NSB), True, True)

    # ---- phase 5: cond half of the output projection ----
    wo_half(1)
    ctx.pop_all().close()


_CACHE = {}


def kernel(x, uc_context, ck, cv, attn_extra, Wq, Wk, Wv, Wo, bo, t):
    global LAST_RESULTS
    x = np.ascontiguousarray(np.asarray(x, np.float32))
    uc_context = np.asarray(uc_context, np.float32)
    ck = np.asarray(ck, np.float32)
    cv = np.asarray(cv, np.float32)
    attn_extra = np.asarray(attn_extra, np.float32)
    Wq = np.asarray(Wq, np.float32)
    Wk = np.asarray(Wk, np.float32)
    Wv = np.asarray(Wv, np.float32)
    Wo = np.asarray(Wo, np.float32)
    bo = np.asarray(bo, np.float32)
    tv = float(np.asarray(t))
    wdotw = W_DOT * (tv / TOTAL_STEP) * SCHED

    if wdotw not in _CACHE:
        _CACHE[wdotw] = build_kernel(wdotw)
    nc = _CACHE[wdotw]

    # host-side input prep (layout only)
    wq_pad = np.zeros((384, INNER), np.float16)
    wq_pad[:DQ] = (Wq * SCALE).astype(np.float16)
    bo_pad = np.zeros((384,), np.float32)
    bo_pad[:DQ] = bo
    wk16 = Wk.astype(np.float16)
    wv16 = Wv.astype(np.float16)
    wo16 = Wo.astype(np.float16)
    ctxK = np.concatenate([uc_context[0][None], ck[:, 0]], axis=0)  # [5, 77, 768]
    ctxV = np.concatenate([uc_context[0][None], cv[:, 0]], axis=0)
    ctxkt = np.ascontiguousarray(ctxK.transpose(2, 0, 1).reshape(DC, 5 * L)).astype(np.float16)
    ctxvt = np.ascontiguousarray(ctxV.transpose(2, 0, 1).reshape(DC, 5 * L)).astype(np.float16)

    in_maps = []
    for c in range(N_CORES):
        rows = slice(c * NS, (c + 1) * NS)
        xt = np.zeros((384, NSB), np.float16)
        xt[:DQ, :NS] = x[0, rows].T.astype(np.float16)
        xt[:DQ, NS:] = x[1, rows].T.astype(np.float16)
        aet = np.ascontiguousarray(
            attn_extra[:, rows, :].transpose(0, 2, 1)).astype(np.float16)
        in_maps.append({
            "xt": xt, "wq": wq_pad, "wk": wk16, "wv": wv16, "wo": wo16, "bo": bo_pad,
            "ctxkt": ctxkt, "ctxvt": ctxvt, "aet": aet,
        })

    import os as _os
    _tc = None
    if _os.environ.get("KERNEL_TRACE_ALL") == "1":
        _tc = list(range(N_CORES))
    res = bass_utils.run_bass_kernel_spmd(
        nc, in_maps, core_ids=list(range(N_CORES)), trace=TRACE, trace_cores=_tc,
    )
    LAST_RESULTS = res

    out = np.empty((2, N, DQ), np.float32)
    for c in range(N_CORES):
        rows = slice(c * NS, (c + 1) * NS)
        yt = res.results[c]["yt"].astype(np.float32)
        out[0, rows] = yt[:, :NS].T
        out[1, rows] = yt[:, NS:].T
    return out


# revision 10
# speedup vs baseline: 1.1743x; 1.0572x over previous
"""Trainium2 Bass kernel for the CrossAttention problem (self-contained).

Strategy: shard the N=4096 query rows across 8 cores (512 rows/core, both
batch elements). Everything is computed in transposed layout (features on
partitions, query rows on the free dim) so every matmul has a wide moving
operand:

  qT   = (scale*Wq)^T @ xT          [512, 1024]   (rows 0:512 uc, 512:1024 cond)
  kT   = Wk^T @ ctxKT               [512, 5*77]   (uc, c0..c3 contexts)
  v    = ctxVT_g^T @ Wv             [5][77, 512]
  simT = k_gh @ qT_h                [77, 512] per (group, head)
  E    = exp(simT)  (logits are small; no max-subtraction needed)
  Z    = ones77^T @ E               [77, 512] bcast rows, written back into the
                                    same PSUM tile the sims came from
  attn = RECIP1_MUL(Z, E)           one fused custom-DVE op: E * ~(1/Z)
                                    (exponent-flip seed + 1 Newton step, consts
                                    minimax-tuned; ~1.7e-3 rel err)
  outT = v_gh^T @ attn  (accumulated over the 4 cond branches; uc separate)
  yT   = Wo^T @ out_mergedT + bo    [320, 1024]

The soft-mask scalar wmask = w_dot * (t/50*4.6) * max(sim_c[0]) couples all
cores: each core computes its local branch-0 max, an AllGather collective
combines them while the other 4 groups are processed, then branch 0 finishes.
"""

import sys

sys.path.insert(0, "/opt/trn_rl_repo")

import numpy as np

import concourse.bass as bass
import concourse.tile as tile
from concourse import bacc, bass_utils, mybir
from concourse import dve_ops
from concourse.dve_spec import AluOp, Bin, Spec, Src0, Src1, C0, C1, C2, lower, _has_src1
from concourse.dve_uop import DveOpSpec

# ---- custom fused DVE op: out = Src1 * recip1NR(Src0) * C2 -----------------
_not_x = Bin(AluOp.BITWISE_NOT, Src0, Src0)
_y0 = _not_x * C0
_y1 = _y0 * (C1 - Src0 * _y0)


def _ref_recip1_mul(in0, in1, s0, s1, imm2):
    not_x = (~in0.view(np.int32)).view(np.float32)
    y0 = not_x * s0
    y1 = y0 * (s1 - in0 * y0)
    return in1 * y1


def _register_recip1_mul():
    for op in dve_ops.OPS:
        if op.name == "RECIP1_MUL_ANT":
            return op
    op = dve_ops.DveOp(
        "RECIP1_MUL_ANT",
        Spec(body=Src1 * _y1, reference=_ref_recip1_mul),
        subdim=False,
        uops_sha={},
    )
    dve_ops.OPS.append(op)
    dve_ops._SUB_OPCODE_FOR_NAME[op.name] = (
        dve_ops._CUSTOM_DVE_ROW_BASE + len(dve_ops.OPS) - 1)
    assert max(dve_ops._SUB_OPCODE_FOR_NAME.values()) < 0x20
    for ver in ("v3", "v4"):
        res = DveOpSpec(name=op.name, opcode=dve_ops.get_dve_sub_opcode(op.name),
                        uops=lower(op.spec, ver=ver), rd1_en=_has_src1(op.spec))
        op.uops_sha[ver] = res.sha(ver)
    return op


RECIP1_MUL = _register_recip1_mul()
# minimax constants for 1/x (octave-periodic, range-insensitive); the output
# scale of the 3-param fit is folded into c0/c1 (c' = sqrt(s)*c)
RC0, RC1 = -0.23549776, 2.00173237

# problem constants (hardcoded per the harness contract)
H, DH, L, C = 8, 64, 77, 4
N, DQ, DC, INNER = 4096, 320, 768, 512
N_CORES = 8
NS = N // N_CORES          # query rows per core per batch element
NSB = 2 * NS               # both batch elements
SCALE = DH ** -0.5
W_DOT, TOTAL_STEP, SCHED = 1.0, 50, 4.6

F32 = mybir.dt.float32
F16 = mybir.dt.float16

G_UC = 0                   # groups in context order: 0=uc, 1..4 = cond branches

LAST_RESULTS = None        # BassKernelResults of the most recent run (for test.py)
TRACE = False


def build_kernel(wdotw: float):
    nc = bacc.Bacc("TRN2", target_bir_lowering=False, debug=False, num_devices=N_CORES)

    d_xt = nc.dram_tensor("xt", [384, NSB], F16, kind="ExternalInput")
    d_wq = nc.dram_tensor("wq", [384, INNER], F16, kind="ExternalInput")  # pre-scaled
    d_wk = nc.dram_tensor("wk", [DC, INNER], F16, kind="ExternalInput")
    d_wv = nc.dram_tensor("wv", [DC, INNER], F16, kind="ExternalInput")
    d_wo = nc.dram_tensor("wo", [INNER, DQ], F16, kind="ExternalInput")
    d_bo = nc.dram_tensor("bo", [384], F32, kind="ExternalInput")
    d_ctxkt = nc.dram_tensor("ctxkt", [DC, 5 * L], F16, kind="ExternalInput")
    d_ctxvt = nc.dram_tensor("ctxvt", [DC, 5 * L], F16, kind="ExternalInput")
    d_aet = nc.dram_tensor("aet", [H, L, NS], F16, kind="ExternalInput")
    d_yt = nc.dram_tensor("yt", [DQ, NSB], F16, kind="ExternalOutput")

    with tile.TileContext(nc) as tc:
        _emit(nc, tc, wdotw, d_xt, d_wq, d_wk, d_wv, d_wo, d_bo,
              d_ctxkt, d_ctxvt, d_aet, d_yt)
    nc.compile()
    return nc


def _emit(nc, tc, wdotw, d_xt, d_wq, d_wk, d_wv, d_wo, d_bo,
          d_ctxkt, d_ctxvt, d_aet, d_yt):
    from contextlib import ExitStack

    ctx = ExitStack()
    singles = ctx.enter_context(tc.tile_pool(name="singles", bufs=1))
    dram = ctx.enter_context(tc.tile_pool(name="dram", bufs=1, space="DRAM"))
    epool = ctx.enter_context(tc.tile_pool(name="epool", bufs=22))
    mpool = ctx.enter_context(tc.tile_pool(name="mpool", bufs=3))

    # ---- persistent SBUF tiles ----
    s_xt = singles.tile([128, 3, NSB], F16)
    s_wq = singles.tile([128, 3, INNER], F16)
    s_wk = singles.tile([128, 6, INNER], F16)
    s_wv = singles.tile([128, 6, INNER], F16)
    s_wo = singles.tile([128, 4, DQ], F16)
    s_bo = singles.tile([128, 3], F32)
    s_ctxkt = singles.tile([128, 6, 5 * L], F16)
    s_ctxvt = singles.tile([128, 6, 5 * L], F16)
    s_aet = singles.tile([L, H, NS], F16)
    s_qt = singles.tile([128, 4, NSB], F16)
    s_kt = singles.tile([128, 4, 5 * L], F16)
    s_vp = singles.tile([L, 5, INNER], F16)
    s_sc0 = singles.tile([L, H, NS], F16)       # branch-0 sims parked pre-mask
    s_om = singles.tile([128, 4, NSB], F16)     # merged outT (inner on partitions)
    s_y = singles.tile([128, 3, NSB], F16)
    s_lmax = singles.tile([L, H], F32)
    s_lm = singles.tile([L, 1], F32)
    s_maxrow8 = singles.tile([1, N_CORES * L], F32)
    s_wm = singles.tile([1, 1], F16)
    s_wmcol = singles.tile([L, 1], F32)
    ones77 = singles.tile([L, L], F16)
    ones_row = singles.tile([1, 128], F16)

    # ---- critical-path input DMA (collective prerequisites first) ----
    nc.sync.dma_start(out=s_xt[:], in_=d_xt.ap().rearrange("(c p) f -> p c f", p=128))
    nc.sync.dma_start(out=s_wq[:], in_=d_wq.ap().rearrange("(c p) f -> p c f", p=128))
    nc.sync.dma_start(out=s_wk[:], in_=d_wk.ap().rearrange("(c p) f -> p c f", p=128))
    nc.sync.dma_start(out=s_ctxkt[:], in_=d_ctxkt.ap().rearrange("(c p) f -> p c f", p=128))

    nc.vector.memset(ones77[:], 1.0)
    nc.vector.memset(ones_row[:], 1.0)

    # PSUM: psim bufs=3 x [77,2,512]f32 (2 banks each) + pgen bufs=2 x
    # [128,512]f32 (1 bank each) = exactly 8 banks.
    psim = ctx.enter_context(tc.tile_pool(name="psim", bufs=3, space="PSUM"))
    pgen = ctx.enter_context(tc.tile_pool(name="pgen", bufs=2, space="PSUM"))

    # The NRT collective barrier is triggered off the Tensor queue; without
    # this DMA-independent kick the first matmul (and thus the barrier) waits
    # ~20us for input DMA, serializing the wmask AllGather behind phase 3.
    p_kick = pgen.tile([128, NS], F32, tag="gen")
    nc.tensor.matmul(p_kick[0:1, 0:1], ones_row[0:1, 0:1], ones_row[0:1, 0:1],
                     start=True, stop=True)

    def qproj(half):
        for dc in range(4):
            p = pgen.tile([128, NS], F32, tag="gen")
            for kc in range(3):
                nc.tensor.matmul(
                    p[:],
                    s_wq[:, kc, dc * 128:(dc + 1) * 128],
                    s_xt[:, kc, half * NS:(half + 1) * NS],
                    start=(kc == 0), stop=(kc == 2),
                )
            nc.scalar.copy(s_qt[:, dc, half * NS:(half + 1) * NS], p[:])

    # ---- phase 0: cond qT, full kT, branch-0 sims -> local max -> collective
    qproj(1)
    for dc in range(4):
        p = pgen.tile([128, NS], F32, tag="gen")
        for kc in range(6):
            nc.tensor.matmul(
                p[0:128, 0:5 * L],
                s_wk[:, kc, dc * 128:(dc + 1) * 128],
                s_ctxkt[:, kc, :],
                start=(kc == 0), stop=(kc == 5),
            )
        nc.scalar.copy(s_kt[:, dc, :], p[0:128, 0:5 * L])

    def qk(g, h, psum_slice):
        cols = slice(0, NS) if g == G_UC else slice(NS, NSB)
        nc.tensor.matmul(
            psum_slice,
            s_kt[(h % 2) * 64:(h % 2) * 64 + 64, h // 2, g * L:(g + 1) * L],
            s_qt[(h % 2) * 64:(h % 2) * 64 + 64, h // 2, cols],
            start=True, stop=True,
        )

    for hp in range(4):
        p = psim.tile([L, 2, NS], F32, tag="sim")
        qk(1, 2 * hp, p[:, 0, :])
        qk(1, 2 * hp + 1, p[:, 1, :])
        nc.vector.reduce_max(out=s_lmax[:, 2 * hp:2 * hp + 2], in_=p[:],
                             axis=mybir.AxisListType.X)
        nc.scalar.copy(s_sc0[:, 2 * hp:2 * hp + 2, :], p[:])
    nc.vector.reduce_max(out=s_lm[:], in_=s_lmax[:], axis=mybir.AxisListType.X)
    nc.vector.tensor_scalar_mul(s_lm[:], s_lm[:], float(wdotw))

    cin = dram.tile([1, L], F32)
    cout = dram.tile([N_CORES, L], F32)
    nc.sync.dma_start(out=cin.rearrange("one f -> f one"), in_=s_lm[:])
    nc.gpsimd.collective_compute(
        "AllGather", mybir.AluOpType.bypass,
        replica_groups=[list(range(N_CORES))],
        ins=[cin.opt()], outs=[cout.opt()],
    )

    # ---- remaining input DMA ----
    nc.sync.dma_start(out=s_wv[:], in_=d_wv.ap().rearrange("(c p) f -> p c f", p=128))
    nc.sync.dma_start(out=s_ctxvt[:], in_=d_ctxvt.ap().rearrange("(c p) f -> p c f", p=128))
    nc.sync.dma_start(out=s_wo[:], in_=d_wo.ap().rearrange("(c p) f -> p c f", p=128))
    nc.sync.dma_start(out=s_bo[:], in_=d_bo.ap().rearrange("(c p) -> p c", p=128))
    nc.sync.dma_start(out=s_aet[:], in_=d_aet.ap().rearrange("h p f -> p h f"))

    # ---- phase 1: uc qT + v projections ----
    qproj(0)
    for g in range(5):                           # v, with 1/C folded into cond
        p = pgen.tile([128, NS], F32, tag="gen")
        for kc in range(6):
            nc.tensor.matmul(
                p[0:L, :],
                s_ctxvt[:, kc, g * L:(g + 1) * L],
                s_wv[:, kc, :],
                start=(kc == 0), stop=(kc == 5),
            )
        if g == G_UC:
            nc.scalar.copy(s_vp[:, g, :], p[0:L, :])
        else:
            nc.scalar.mul(s_vp[:, g, :], p[0:L, :], 1.0 / C)

    # ---- phase 3: per-pair pipeline for groups uc, c1, c2, c3 -------------
    # Per pair: QK (PE, into psim slot) -> exp (ACT, psim->e f16) -> Z-matmul
    # (PE, ones77 @ e, written back INTO the same psim slot) -> fused
    # recip-mul (DVE, attn overwrites e in place).  QK runs 2 pairs ahead of
    # Z so the tensor engine never waits on the softmax chain.
    attn_c = {}
    anchors = {}

    def pair_front(g, hp):
        """QK for pair hp of group g -> returns psim tile."""
        p = psim.tile([L, 2, NS], F32, tag="sim")
        qk(g, 2 * hp, p[:, 0, :])
        qk(g, 2 * hp + 1, p[:, 1, :])
        return p

    def pair_mid(g, hp, p):
        """exp -> Z (back into p) -> fused recip-mul; returns attn (f16)."""
        e = epool.tile([L, 2, NS], F16, tag="e")
        anchors["exp"] = nc.scalar.activation(
            e[:], p[:], mybir.ActivationFunctionType.Exp)
        for k in range(2):
            anchors["z"] = nc.tensor.matmul(
                p[:, k, :], ones77[:], e[:, k, :], start=True, stop=True)
        anchors["rm"] = nc.vector._custom_dve(
            RECIP1_MUL, out=e[:], in0=p[:], in1=e[:], s0=RC0, s1=RC1)
        return e

    # s_om layout matches the baseline merge: head h occupies partitions
    # (h%2)*64..+64 of free-chunk h//2, so pair (2hp, 2hp+1) is exactly the
    # full 128 partitions of chunk hp — one copy per pair.
    def pv_pair(g_list, hp, a_by_g, cols, start, stop):
        pv = pgen.tile([128, NS], F32, tag="gen")
        for k in range(2):
            h = 2 * hp + k
            for i, g in enumerate(g_list):
                nc.tensor.matmul(pv[k * 64:k * 64 + 64, :],
                                 s_vp[:, g, h * 64:(h + 1) * 64],
                                 a_by_g[g][:, k, :],
                                 start=(i == 0), stop=(i == len(g_list) - 1))
        nc.scalar.copy(s_om[:, hp, cols], pv[:])

    # uc group first (PV immediate), then cond groups 2..4 (attn parked)
    fronts = {}
    order = [(0, hp) for hp in range(4)] + [(g, hp) for g in (2, 3, 4) for hp in range(4)]
    lead = 2
    for i, (g, hp) in enumerate(order):
        fronts[(g, hp)] = pair_front(g, hp)
        j = i - lead
        if j >= 0:
            gj, hpj = order[j]
            a = pair_mid(gj, hpj, fronts.pop((gj, hpj)))
            if gj == 0:
                pv_pair([0], hpj, {0: a}, slice(0, NS), True, True)
            else:
                attn_c[(gj, hpj)] = a
    for j in range(len(order) - lead, len(order)):
        gj, hpj = order[j]
        a = pair_mid(gj, hpj, fronts.pop((gj, hpj)))
        if gj == 0:
            pv_pair([0], hpj, {0: a}, slice(0, NS), True, True)
        else:
            attn_c[(gj, hpj)] = a

    # ---- uc half of the output projection (fills PE while collective lands)
    def wo_half(half):
        for oc in range(3):
            ow = 128 if oc < 2 else 64
            p = pgen.tile([128, NS], F32, tag="gen")
            for kc in range(4):
                nc.tensor.matmul(
                    p[0:ow, :],
                    s_wo[:, kc, oc * 128:oc * 128 + ow],
                    s_om[:, kc, half * NS:(half + 1) * NS],
                    start=(kc == 0), stop=(kc == 3),
                )
            nc.scalar.add(s_y[0:ow, oc, half * NS:(half + 1) * NS], p[0:ow, :],
                          s_bo[0:ow, oc:oc + 1])
        for oc in range(3):
            ow = 128 if oc < 2 else 64
            nc.sync.dma_start(
                out=d_yt.ap()[oc * 128:oc * 128 + ow, half * NS:(half + 1) * NS],
                in_=s_y[0:ow, oc, half * NS:(half + 1) * NS])

    wo_half(0)

    # ---- phase 4: wmask from the gathered maxima, branch 0, PV chains ----
    nc.sync.dma_start(out=s_maxrow8[:], in_=cout.rearrange("r f -> (r f)"))
    red = nc.vector.reduce_max(out=s_wm[:], in_=s_maxrow8[:], axis=mybir.AxisListType.X)
    tile.add_dep_helper(red.ins, anchors["rm"].ins, sync=False,
                        reason="defer wmask path behind group work")
    p_wm = psim.tile([L, 2, NS], F32, tag="sim")
    bc = nc.tensor.matmul(p_wm[:, 0, 0:1], ones_row[0:1, 0:L], s_wm[:],
                          start=True, stop=True)
    tile.add_dep_helper(bc.ins, anchors["z"].ins, sync=False,
                        reason="defer wmask bcast behind group matmuls")
    nc.vector.tensor_copy(s_wmcol[:], p_wm[:, 0, 0:1])

    b0 = {}
    first_p4_exp = [None]

    def b0_mid(hp):
        msk = mpool.tile([L, 2, NS], F16, tag="msk")
        nc.vector.scalar_tensor_tensor(
            out=msk[:], in0=s_aet[:, 2 * hp:2 * hp + 2, :], scalar=s_wmcol[:],
            in1=s_sc0[:, 2 * hp:2 * hp + 2, :],
            op0=mybir.AluOpType.mult, op1=mybir.AluOpType.add,
        )
        e = epool.tile([L, 2, NS], F16, tag="e")
        ei = nc.scalar.activation(e[:], msk[:], mybir.ActivationFunctionType.Exp)
        if first_p4_exp[0] is None:
            first_p4_exp[0] = ei
            tile.add_dep_helper(ei.ins, anchors["exp"].ins, sync=False,
                                reason="defer branch-0 exp behind group exps")
        p = psim.tile([L, 2, NS], F32, tag="sim")
        for k in range(2):
            nc.tensor.matmul(p[:, k, :], ones77[:], e[:, k, :], start=True, stop=True)
        nc.vector._custom_dve(
            RECIP1_MUL, out=e[:], in0=p[:], in1=e[:], s0=RC0, s1=RC1)
        return e

    for hp in range(4):
        b0[hp] = b0_mid(hp)
    for hp in range(4):
        amap = {1: b0[hp], 2: attn_c.pop((2, hp)), 3: attn_c.pop((3, hp)),
                4: attn_c.pop((4, hp))}
        pv_pair([1, 2, 3, 4], hp, amap, slice(NS, NSB), True, True)

    # ---- phase 5: cond half of the output projection ----
    wo_half(1)
    ctx.pop_all().close()


_CACHE = {}


def kernel(x, uc_context, ck, cv, attn_extra, Wq, Wk, Wv, Wo, bo, t):
    global LAST_RESULTS
    x = np.ascontiguousarray(np.asarray(x, np.float32))
    uc_context = np.asarray(uc_context, np.float32)
    ck = np.asarray(ck, np.float32)
    cv = np.asarray(cv, np.float32)
    attn_extra = np.asarray(attn_extra, np.float32)
    Wq = np.asarray(Wq, np.float32)
    Wk = np.asarray(Wk, np.float32)
    Wv = np.asarray(Wv, np.float32)
    Wo = np.asarray(Wo, np.float32)
    bo = np.asarray(bo, np.float32)
    tv = float(np.asarray(t))
    wdotw = W_DOT * (tv / TOTAL_STEP) * SCHED

    if wdotw not in _CACHE:
        _CACHE[wdotw] = build_kernel(wdotw)
    nc = _CACHE[wdotw]

    # host-side input prep (layout only)
    wq_pad = np.zeros((384, INNER), np.float16)
    wq_pad[:DQ] = (Wq * SCALE).astype(np.float16)
    bo_pad = np.zeros((384,), np.float32)
    bo_pad[:DQ] = bo
    wk16 = Wk.astype(np.float16)
    wv16 = Wv.astype(np.float16)
    wo16 = Wo.astype(np.float16)
    ctxK = np.concatenate([uc_context[0][None], ck[:, 0]], axis=0)  # [5, 77, 768]
    ctxV = np.concatenate([uc_context[0][None], cv[:, 0]], axis=0)
    ctxkt = np.ascontiguousarray(ctxK.transpose(2, 0, 1).reshape(DC, 5 * L)).astype(np.float16)
    ctxvt = np.ascontiguousarray(ctxV.transpose(2, 0, 1).reshape(DC, 5 * L)).astype(np.float16)

    in_maps = []
    for c in range(N_CORES):
        rows = slice(c * NS, (c + 1) * NS)
        xt = np.zeros((384, NSB), np.float16)
        xt[:DQ, :NS] = x[0, rows].T.astype(np.float16)
        xt[:DQ, NS:] = x[1, rows].T.astype(np.float16)
        aet = np.ascontiguousarray(
            attn_extra[:, rows, :].transpose(0, 2, 1)).astype(np.float16)
        in_maps.append({
            "xt": xt, "wq": wq_pad, "wk": wk16, "wv": wv16, "wo": wo16, "bo": bo_pad,
            "ctxkt": ctxkt, "ctxvt": ctxvt, "aet": aet,
        })

    import os as _os
    _tc = None
    if _os.environ.get("KERNEL_TRACE_ALL") == "1":
        _tc = list(range(N_CORES))
    res = bass_utils.run_bass_kernel_spmd(
        nc, in_maps, core_ids=list(range(N_CORES)), trace=TRACE, trace_cores=_tc,
    )
    LAST_RESULTS = res

    out = np.empty((2, N, DQ), np.float32)
    for c in range(N_CORES):
        rows = slice(c * NS, (c + 1) * NS)
        yt = res.results[c]["yt"].astype(np.float32)
        out[0, rows] = yt[:, :NS].T
        out[1, rows] = yt[:, NS:].T
    return out
